# revision 4
# baseline (speedup 1.0000x reference)
"""CPWanSelfAttention on 8 Trainium2 NeuronCores.

Strategy: tensor-parallel over heads (16 heads -> 2 per core), fp16 datapath.
Per core c (heads 2c, 2c+1):
  - qT/kT = wq_c @ hiddenT in transposed-per-head layout [dhead, S], head dims
    host-permuted to [evens..., odds...] so RoPE pair math is half-tile ops.
  - vT natural [S, d] (PV stationary operand).
  - RMS norm: per-core partial sum-of-squares -> AllReduce [1, 2048] per seq
    half -> row-wise sqrt+recip on [1,2048] -> DMA partition-broadcast fp16.
  - Attention split into half-A (key tiles 0-7, available after RoPE of seq
    half 0) and half-B (key tiles 8-15): half-A's ~20us of work bridges the
    second AllReduce's latency so PE never idles on it. Half-A partial PV
    sums spill to SBUF fp16; half-B re-accumulates and finishes.
  - scoresT[k, q] per head with PAIRED key tiles: two 512-row matmuls into
    one [128,1024] PSUM tile, ONE exp per pair (halves ACT instruction
    overhead, the previous ACT bottleneck). Denominator = DVE fp16 adds +
    ones-matmul broadcast; divide at the end.
  - AllGather per-head outputs in fp16 per 512-seq chunk; out projection
    (lag-1) computes the core's 256 output columns; host transposes/concats.
All matmuls fp16 (1 cyc/row, fast weight loads); PSUM accumulates f32.
Measured max |logit| = 7.2 -> exp < 1.4e3 << fp16 max, so no max-subtraction
(faithful to the nki_flash_attention reference, which also omits it).
"""

from contextlib import ExitStack

import numpy as np
import concourse.bass as bass
import concourse.mybir as mybir
import concourse.tile as tile
from concourse import bacc
from concourse.bass_utils import run_bass_kernel_spmd

N_CORES = 8
S = 1992
SP = 2048          # seq padded to multiple of 128 (nki flash attention contract)
DIM = 2048
NHEADS = 16
DH = 128
HPC = NHEADS // N_CORES   # heads per core = 2
DC = DH * HPC             # out dims per core = 256
KT = DIM // 128           # 16 contraction tiles
NCH = SP // 512           # 4 seq chunks of 512
EPS = 1e-6

F32 = mybir.dt.float32
F16 = mybir.dt.float16

_COMPILED = None


def _build(ag_mode='chunk4', repeat=1, stage=4):
    nc = bacc.Bacc("TRN2", target_bir_lowering=False, debug=False,
                   num_devices=N_CORES)

    # ---- DRAM I/O (per-core shards) ----
    hidT = nc.dram_tensor("hidT", [SP // 512, 128, KT, 512], F16, kind="ExternalInput")
    wqT = nc.dram_tensor("wqT", [128, KT, DC], F16, kind="ExternalInput")
    wkT = nc.dram_tensor("wkT", [128, KT, DC], F16, kind="ExternalInput")
    wvT = nc.dram_tensor("wvT", [128, KT, DC], F16, kind="ExternalInput")
    woT = nc.dram_tensor("woT", [128, KT, DC], F16, kind="ExternalInput")
    cosT = nc.dram_tensor("cosT", [DH, SP], F16, kind="ExternalInput")  # [c;c]
    sinT = nc.dram_tensor("sinT", [DH, SP], F16, kind="ExternalInput")  # [-s;s]
    bq = nc.dram_tensor("bq", [HPC, DH], F32, kind="ExternalInput")
    bk = nc.dram_tensor("bk", [HPC, DH], F32, kind="ExternalInput")
    bv = nc.dram_tensor("bv", [1, DC], F16, kind="ExternalInput")
    bo = nc.dram_tensor("bo", [HPC, DH], F32, kind="ExternalInput")
    nwq = nc.dram_tensor("nwq", [HPC, DH], F32, kind="ExternalInput")
    nwk = nc.dram_tensor("nwk", [HPC, DH], F32, kind="ExternalInput")
    outT = nc.dram_tensor("outT", [DC, S], F32, kind="ExternalOutput")

    rg = [list(range(N_CORES))]
    inv_sqrt_dh = 1.0 / float(np.sqrt(DH))

    def emit(tc, top, rep):
        P = lambda nm: f"{nm}_{rep}"
        const = top.enter_context(tc.tile_pool(name=P("const"), bufs=1))
        pv_pool = top.enter_context(tc.tile_pool(name=P("pv_pool"), bufs=1))
        dram = top.enter_context(tc.tile_pool(name=P("dram"), bufs=1, space="DRAM"))

        ones_col = const.tile([128, 1], F16)
        nc.vector.memset(ones_col[:], 1.0)
        ones_sq = const.tile([128, 128], F16)
        nc.vector.memset(ones_sq[:], 1.0)
        ones_row = const.tile([1, 128], F16)
        nc.vector.memset(ones_row[:], 1.0)
        bq_sb = const.tile([128, HPC], F32)
        bk_sb = const.tile([128, HPC], F32)
        bo_sb = const.tile([128, HPC], F32)
        nwq_sb = const.tile([128, HPC], F32)
        nwk_sb = const.tile([128, HPC], F32)
        # DMA issue occupies the issuing engine's queue (only SP/ACT/GpSimd
        # can trigger); spread the preamble so SP isn't a 20us serial
        # bottleneck. GpSimd is idle until the first collective at ~65us.
        nc.gpsimd.dma_start(bq_sb[:], bq[:].rearrange("h p -> p h"))
        nc.gpsimd.dma_start(bk_sb[:], bk[:].rearrange("h p -> p h"))
        bv_sb = const.tile([1, DC], F16)
        eps_sb = const.tile([128, 1], F32)
        nc.vector.memset(eps_sb[:], EPS)
        # RoPE tables + wo loaded up front (consts; overlap phase-1 compute)
        cos_sb = const.tile([DH, SP], F16)
        sin_sb = const.tile([DH, SP], F16)
        nc.scalar.dma_start(cos_sb[:, 0:SP // 2], cosT[:, 0:SP // 2])
        nc.scalar.dma_start(cos_sb[:, SP // 2:SP], cosT[:, SP // 2:SP])
        nc.scalar.dma_start(sin_sb[:, 0:SP // 2], sinT[:, 0:SP // 2])
        nc.scalar.dma_start(sin_sb[:, SP // 2:SP], sinT[:, SP // 2:SP])
        wo_sb = const.tile([128, KT, DC], F16)

        v_sb = pv_pool.tile([128, SP // 128, DC], F16)  # [s%128, s-tile, d]
        late = top.enter_context(tc.tile_pool(name=P("late"), bufs=1))
        qT = [late.tile([128, SP], F16, name=f"qT{h}_{rep}") for h in range(HPC)]
        kTt = [late.tile([128, SP], F16, name=f"kTt{h}_{rep}") for h in range(HPC)]

        # 2 AllReduce halves over seq: half m covers s in [m*1024,(m+1)*1024);
        # within a half: cols [0:1024] = q sumsq, [1024:2048] = k sumsq
        ar_in = [dram.tile([1, SP], F32, name=f"ar_in{m}_{rep}") for m in range(2)]
        ar_out = [dram.tile([1, SP], F32, addr_space="Shared", name=f"ar_out{m}_{rep}")
                  for m in range(2)]
        rbounce = [dram.tile([1, SP], F16, name=f"rb{m}_{rep}") for m in range(2)]

        with ExitStack() as ph123:
            rawp = ph123.enter_context(tc.tile_pool(name=P("rawp"), bufs=1))
            qraw = [rawp.tile([128, SP], F16, name=f"qraw{h}_{rep}") for h in range(HPC)]
            kraw = [rawp.tile([128, SP], F16, name=f"kraw{h}_{rep}") for h in range(HPC)]

            stat = ph123.enter_context(tc.tile_pool(name=P("stat"), bufs=1))
            rstd_bc = [stat.tile([128, SP], F16, name=f"rstdbc{i}_{rep}")
                       for i in range(2)]
            strip = ph123.enter_context(tc.tile_pool(name=P("strip"), bufs=2))
            rwork = ph123.enter_context(tc.tile_pool(name=P("rwork"), bufs=1))

            def rstd_prep(m):
                """rstd = exp(-0.5 * ln(ms + eps)) on a [16,128] reshape of the
                AllReduce row (16 partitions -> cheap DMA), then fp16
                partition-broadcast via a DRAM bounce. Ln and Exp live in the
                same ACT table set as Identity, so the whole kernel needs ONE
                activation-table load - no reload stalls on this chain."""
                srow = strip.tile([16, 128], F32, name="srow")
                nc.sync.dma_start(
                    srow[:], ar_out[m][:].rearrange("o (p c) -> (o p) c", p=16))
                nc.scalar.activation(srow[:], srow[:],
                                     mybir.ActivationFunctionType.Ln,
                                     bias=eps_sb[0:16, :], scale=1.0 / DIM)
                rrow = strip.tile([16, 128], F16, name="rrow")
                nc.scalar.activation(rrow[:], srow[:],
                                     mybir.ActivationFunctionType.Exp,
                                     scale=-0.5)
                nc.sync.dma_start(
                    rbounce[m][:].rearrange("o (p c) -> (o p) c", p=16), rrow[:])
                sj = slice(m * 1024, (m + 1) * 1024)
                for i in (1, 0):                        # k first: rope needs it
                    nc.sync.dma_start(
                        rstd_bc[i][:, sj],
                        rbounce[m][:, i * 1024:(i + 1) * 1024]
                        .partition_broadcast(128))

            def rope_one(raw, dst, rbc, nw, h, m):
                """norm + RoPE for seq half m (1024 wide), all fp16."""
                sj = slice(m * 1024, (m + 1) * 1024)
                xr = rwork.tile([128, 1024], F16, name="xr")
                nc.vector.tensor_mul(xr[:], raw[h][:, sj], rbc[:, sj])
                nc.vector.tensor_scalar_mul(xr[:], xr[:], nw[:, h:h + 1])
                xs = rwork.tile([128, 1024], F16, name="xs")
                nc.vector.tensor_copy(xs[0:64, :], xr[64:128, :])
                nc.vector.tensor_copy(xs[64:128, :], xr[0:64, :])
                nc.vector.tensor_mul(xr[:], xr[:], cos_sb[:, sj])
                nc.vector.tensor_mul(xs[:], xs[:], sin_sb[:, sj])
                nc.vector.tensor_add(dst[h][:, sj], xr[:], xs[:])
                if m == 1:
                    nc.vector.memset(dst[h][:, S:SP], 0.0)

            # ---------- phase 1: QKV + sumsq per 512-chunk, ARs per half ----
            with ExitStack() as ph1:
                wpool = ph1.enter_context(tc.tile_pool(name=P("wpool"), bufs=1))
                hid = ph1.enter_context(tc.tile_pool(name=P("hid"), bufs=3))
                psA = ph1.enter_context(tc.tile_pool(name=P("psA"), bufs=2, space="PSUM"))

                wq_sb = wpool.tile([128, KT, DC], F16)
                wk_sb = wpool.tile([128, KT, DC], F16)
                wv_sb = wpool.tile([128, KT, DC], F16)
                # first-needed operands on otherwise-idle engine queues
                nc.scalar.dma_start(wq_sb[:, 0:KT // 2, :], wqT[:, 0:KT // 2, :])
                nc.scalar.dma_start(wq_sb[:, KT // 2:KT, :], wqT[:, KT // 2:KT, :])
                nc.gpsimd.dma_start(wk_sb[:, 0:KT // 2, :], wkT[:, 0:KT // 2, :])
                nc.gpsimd.dma_start(wk_sb[:, KT // 2:KT, :], wkT[:, KT // 2:KT, :])
                nc.gpsimd.dma_start(wv_sb[:, 0:KT // 2, :], wvT[:, 0:KT // 2, :])
                nc.gpsimd.dma_start(wv_sb[:, KT // 2:KT, :], wvT[:, KT // 2:KT, :])
                nc.gpsimd.dma_start(bo_sb[:], bo[:].rearrange("h p -> p h"))
                nc.gpsimd.dma_start(nwq_sb[:], nwq[:].rearrange("h p -> p h"))
                nc.gpsimd.dma_start(nwk_sb[:], nwk[:].rearrange("h p -> p h"))
                nc.gpsimd.dma_start(bv_sb[:], bv[:])
                nc.gpsimd.dma_start(wo_sb[:, 0:KT // 2, :], woT[:, 0:KT // 2, :])
                nc.gpsimd.dma_start(wo_sb[:, KT // 2:KT, :], woT[:, KT // 2:KT, :])

                def v_block(j, hch):
                    # v: natural layout, hiddenT as stationary; bias K=1 matmul
                    for st in range(4):
                        gst = j * 4 + st
                        pvp = psA.tile([128, DC], F32, name="pvp")
                        for t in range(KT):
                            nc.tensor.matmul(
                                pvp[:], hch[:, t, st * 128:(st + 1) * 128],
                                wv_sb[:, t, :], start=(t == 0), stop=False)
                        nc.tensor.matmul(pvp[:], ones_row[:], bv_sb[:],
                                         start=False, stop=True)
                        if gst == SP // 128 - 1:
                            nc.vector.memset(v_sb[:, gst, :], 0.0)
                            nv = S - (SP // 128 - 1) * 128
                            nc.vector.tensor_copy(v_sb[0:nv, gst, :], pvp[0:nv, :])
                        else:
                            nc.vector.tensor_copy(v_sb[:, gst, :], pvp[:])

                hchs = {}
                for j in range(SP // 512):
                    sj = slice(j * 512, (j + 1) * 512)
                    hch = hid.tile([128, KT, 512], F16, name="hch")
                    hchs[j] = hch
                    nc.sync.dma_start(hch[:, 0:KT // 2, :], hidT[j][:, 0:KT // 2, :])
                    nc.sync.dma_start(hch[:, KT // 2:KT, :], hidT[j][:, KT // 2:KT, :])

                    for (wsb, raw, bias) in ((wq_sb, qraw, bq_sb), (wk_sb, kraw, bk_sb)):
                        for h in range(HPC):
                            pq = psA.tile([128, 512], F32, name="pqk")
                            for t in range(KT):
                                nc.tensor.matmul(
                                    pq[:], wsb[:, t, h * DH:(h + 1) * DH],
                                    hch[:, t, :], start=(t == 0), stop=(t == KT - 1))
                            nc.scalar.activation(
                                raw[h][:, sj], pq[:],
                                mybir.ActivationFunctionType.Identity,
                                bias=bias[:, h:h + 1])

                    # partial sum-of-squares -> AllReduce input strip
                    for idx, raw in ((0, qraw), (1, kraw)):
                        pss = psA.tile([1, 512], F32, name="pss")
                        for h in range(HPC):
                            sq = rwork.tile([128, 512], F16, name="sq")
                            nc.vector.tensor_mul(sq[:], raw[h][:, sj], raw[h][:, sj])
                            nc.tensor.matmul(pss[:], ones_col[:], sq[:],
                                             start=(h == 0), stop=(h == HPC - 1))
                        ssv = rwork.tile([1, 512], F32, name="ssv")
                        nc.vector.tensor_copy(ssv[:], pss[:])
                        m, off = j // 2, (j % 2) * 512
                        nc.sync.dma_start(
                            ar_in[m][:, idx * 1024 + off: idx * 1024 + off + 512],
                            ssv[:])

                    # v for the first two chunks only; chunks 2-3 are deferred
                    # below so their PE work (and half-A's) fills the second
                    # AllReduce's latency window instead of idling on it
                    if j < 2:
                        v_block(j, hch)

                    # issue AllReduce for a finished seq-half; its latency is
                    # bridged by remaining phase-1 chunks (half 0) or by
                    # half-A attention below (half 1)
                    if (j == 1 or j == SP // 512 - 1) and stage >= 2:
                        m = j // 2
                        if ag_mode != 'nocoll':
                            nc.gpsimd.collective_compute(
                                "AllReduce", mybir.AluOpType.add, replica_groups=rg,
                                ins=[ar_in[m][:].opt()], outs=[ar_out[m][:].opt()])
                        else:
                            nc.sync.dma_start(ar_out[m][:], ar_in[m][:])

                    # head-0 rstd+rope for seq half 0 emitted mid-chunk-3 so
                    # the DVE finishes it right as phase-1 PE drains and
                    # half-A attention can start immediately (its deps are
                    # only half-0 tiles; v copies of chunk 3 feed half-B only)
                    if j == SP // 512 - 1 and stage >= 2:
                        rstd_prep(0)
                        rope_one(kraw, kTt, rstd_bc[1], nwk_sb, 0, 0)
                        rope_one(qraw, qT, rstd_bc[0], nwq_sb, 0, 0)

                v_block(2, hchs[2])
                v_block(3, hchs[3])
                if stage >= 2:
                    rope_one(kraw, kTt, rstd_bc[1], nwk_sb, 1, 0)
                    rope_one(qraw, qT, rstd_bc[0], nwq_sb, 1, 0)

            if stage < 3:
                return

            # ------- attention halves + chunked AllGather + projection ------
            with ExitStack() as ph4:
                aw = ph4.enter_context(tc.tile_pool(name=P("aw"), bufs=2))
                oT = [aw.tile([128, SP], F16, name=f"oT{h}_{rep}", bufs=1)
                      for h in range(HPC)]
                accp = ph4.enter_context(tc.tile_pool(name=P("accp"), bufs=1))
                spillp = ph4.enter_context(tc.tile_pool(name=P("spillp"), bufs=1))
                expp = ph4.enter_context(tc.tile_pool(name=P("expp"), bufs=6))
                psC = ph4.enter_context(tc.tile_pool(name=P("psC"), bufs=2, space="PSUM"))
                psD = ph4.enter_context(tc.tile_pool(name=P("psD"), bufs=2, space="PSUM"))

                # per (chunk j, head h) state carried from half-A to half-B
                accs = {}    # fp16 [128, 1024] running exp sums (pair-layout)
                spills = {}  # fp16 [128, 512] half-A partial PV sums

                def att_half(j, h, kp_lo, kp_hi):
                    """Score+exp+PV for key-tile pairs [kp_lo, kp_hi) of
                    chunk j (512 q), head h. Pairs: kp covers kt 2kp, 2kp+1."""
                    sj = slice(j * 512, (j + 1) * 512)
                    first = kp_lo == 0
                    po = psD.tile([128, 512], F32, name="po")
                    if first:
                        acc = accp.tile([128, 1024], F16, name=f"acc{j}{h}_{rep}")
                        accs[(j, h)] = acc
                    else:
                        acc = accs[(j, h)]
                    for kp in range(kp_lo, kp_hi):
                        ps2 = psC.tile([128, 1024], F32, name="ps2")
                        for z in range(2):
                            kt = 2 * kp + z
                            nc.tensor.matmul(
                                ps2[:, z * 512:(z + 1) * 512],
                                kTt[h][:, kt * 128:(kt + 1) * 128],
                                qT[h][:, sj], start=True, stop=True)
                        et = expp.tile([128, 1024], F16, name="et")
                        nc.scalar.activation(et[:], ps2[:],
                                             mybir.ActivationFunctionType.Exp,
                                             scale=inv_sqrt_dh)
                        for z in range(2):
                            kt = 2 * kp + z
                            nc.tensor.matmul(
                                po[:], v_sb[:, kt, h * DH:(h + 1) * DH],
                                et[:, z * 512:(z + 1) * 512],
                                start=(kp == kp_lo and z == 0),
                                stop=(kp == kp_hi - 1 and z == 1))
                        # running denominator on the otherwise-idle DVE
                        if kp == 0:
                            nc.vector.tensor_copy(acc[:], et[:])
                        else:
                            nc.vector.tensor_add(acc[:], acc[:], et[:])
                    return po

                def att_A(j, h):
                    po = att_half(j, h, 0, KT // 4)
                    sp = spillp.tile([128, 512], F16, name=f"sp{j}{h}_{rep}")
                    with nc.allow_low_precision(
                            reason="partial PV sums; fp16 matches datapath"):
                        nc.vector.tensor_copy(sp[:], po[:])
                    spills[(j, h)] = sp

                def att_B(j, h):
                    sj = slice(j * 512, (j + 1) * 512)
                    lo = 0 if (j, h) not in spills else KT // 4
                    po = att_half(j, h, lo, KT // 2)
                    acc = accs[(j, h)]
                    psm = psD.tile([128, 512], F32, name="psm", bufs=1)
                    for z in range(2):
                        nc.tensor.matmul(psm[:], ones_sq[:],
                                         acc[:, z * 512:(z + 1) * 512],
                                         start=(z == 0), stop=(z == 1))
                    rec = aw.tile([128, 512], F32, name="rec")
                    nc.vector.reciprocal(rec[:], psm[:])
                    if (j, h) in spills:
                        pf = aw.tile([128, 512], F32, name="pf")
                        nc.vector.tensor_add(pf[:], po[:], spills[(j, h)][:])
                        nc.vector.tensor_mul(oT[h][:, sj], pf[:], rec[:])
                    else:
                        nc.vector.tensor_mul(oT[h][:, sj], po[:], rec[:])

                agos = []

                def issue_ag(j):
                    sj = slice(j * 512, (j + 1) * 512)
                    agi = dram.tile([DC, 512], F16, name=f"agi{j}_{rep}")
                    ago = dram.tile([DIM, 512], F16, addr_space="Shared",
                                    name=f"ago{j}_{rep}")
                    for h in range(HPC):
                        nc.sync.dma_start(agi[h * DH:(h + 1) * DH, :], oT[h][:, sj])
                    if ag_mode != 'nocoll':
                        nc.gpsimd.collective_compute(
                            "AllGather", mybir.AluOpType.bypass, replica_groups=rg,
                            ins=[agi[:].opt()], outs=[ago[:].opt()])
                    agos.append(ago)

                def project(j):
                    ago = agos[j]
                    och = aw.tile([128, KT, 512], F16, name="och")
                    for q4 in range(4):
                        nc.sync.dma_start(
                            och[:, q4 * 4:(q4 + 1) * 4, :],
                            ago[q4 * 512:(q4 + 1) * 512, :]
                            .rearrange("(t p) s -> p t s", p=128))
                    for h in range(HPC):
                        pout = psD.tile([128, 512], F32, name="pout", bufs=1)
                        for t in range(KT):
                            nc.tensor.matmul(
                                pout[:], wo_sb[:, t, h * DH:(h + 1) * DH],
                                och[:, t, :], start=(t == 0), stop=(t == KT - 1))
                        ot = aw.tile([128, 512], F32, name="ot")
                        nc.scalar.activation(ot[:], pout[:],
                                             mybir.ActivationFunctionType.Identity,
                                             bias=bo_sb[:, h:h + 1])
                        w = min(512, S - j * 512)
                        nc.sync.dma_start(
                            outT[h * DH:(h + 1) * DH, j * 512:j * 512 + w],
                            ot[:, 0:w])

                # half-A: chunks 0,1 (seq-half-0 queries) x key tiles 0-7.
                # Emitted before anything that depends on AllReduce half 1, so
                # every engine stream has AR1-independent work to chew on
                # while AR1 + rstd(1) + rope(m=1) complete.
                for h in range(HPC):
                    for j in (0, 1):
                        att_A(j, h)

                rstd_prep(1)
                for h in range(HPC):
                    rope_one(kraw, kTt, rstd_bc[1], nwk_sb, h, 1)
                for h in range(HPC):
                    rope_one(qraw, qT, rstd_bc[0], nwq_sb, h, 1)

                # half-B j-major so each chunk's AllGather issues as early as
                # possible; projections slot into the ACT-bound attention
                # stream's PE slack (lag-1 wrt AllGather completion)
                do_ag = stage >= 4 or ag_mode == 'nocoll'
                att_B(0, 0)
                att_B(0, 1)
                if do_ag:
                    issue_ag(0)
                att_B(1, 0)
                att_B(1, 1)
                if do_ag:
                    issue_ag(1)
                att_B(2, 0)
                att_B(2, 1)
                if do_ag:
                    issue_ag(2)
                if stage >= 4:
                    project(0)
                att_B(3, 0)
                if stage >= 4:
                    project(1)
                att_B(3, 1)
                if do_ag:
                    issue_ag(3)
                if stage >= 4:
                    project(2)
                    project(3)

    with tile.TileContext(nc) as tc:
        for rep in range(repeat):
            with ExitStack() as top:
                emit(tc, top, rep)

    nc.compile()
    return nc


def _prep_inputs(hidden_states, freqs_cos, freqs_sin, wq, bq, wk, bk, wv, bv,
                 norm_q_w, norm_k_w, wo, bo):
    """Host-side shard + layout prep. Returns in_maps for 8 cores."""
    f32 = np.float32
    f16 = np.float16
    hid = np.ascontiguousarray(np.asarray(hidden_states)[0].T, dtype=f32)
    hidT = np.zeros((DIM, SP), dtype=f32)
    hidT[:, :S] = hid
    # pre-tile to [chunk j, partition p, ktile t, col c]: d = t*128+p, s = j*512+c
    hidT = np.ascontiguousarray(
        hidT.reshape(KT, 128, SP // 512, 512).transpose(2, 1, 0, 3)).astype(f16)

    def tile_w(wT, dt=f16):               # [DIM, DC] -> [128, KT, DC]
        return np.ascontiguousarray(
            wT.reshape(KT, 128, DC).transpose(1, 0, 2)).astype(dt)

    # RoPE tables: c_j[s] = cos[0,s,0,2j], s_j[s] = sin[0,s,0,2j+1]; stack [t;t]
    c = np.asarray(freqs_cos)[0, :, 0, 0::2].astype(f32).T          # [64, S]
    s = np.asarray(freqs_sin)[0, :, 0, 1::2].astype(f32).T          # [64, S]
    cosT = np.zeros((DH, SP), dtype=f32)
    sinT = np.zeros((DH, SP), dtype=f32)
    cosT[0:64, :S] = c
    cosT[64:128, :S] = c
    sinT[0:64, :S] = -s
    sinT[64:128, :S] = s
    cosT = cosT.astype(f16)
    sinT = sinT.astype(f16)

    perm = np.concatenate([np.arange(0, DH, 2), np.arange(1, DH, 2)])
    wq = np.asarray(wq)
    wk = np.asarray(wk)
    wv = np.asarray(wv)
    wo = np.asarray(wo)
    bqv = np.asarray(bq)
    bkv = np.asarray(bk)
    bvv = np.asarray(bv)
    bov = np.asarray(bo)
    nq = np.asarray(norm_q_w)
    nk = np.asarray(norm_k_w)

    in_maps = []
    for core in range(N_CORES):
        rows = slice(core * DC, (core + 1) * DC)

        def permuted(mat_rows):                                     # [DC, DIM]
            blocks = [mat_rows[h * DH:(h + 1) * DH][perm] for h in range(HPC)]
            return np.concatenate(blocks, axis=0)

        def permuted_vec(vec_rows):                                 # [HPC, DH]
            blocks = [vec_rows[h * DH:(h + 1) * DH][perm] for h in range(HPC)]
            return np.stack(blocks, axis=0)

        wq_c = permuted(wq[rows].astype(f32))
        wk_c = permuted(wk[rows].astype(f32))
        in_maps.append({
            "hidT": hidT,
            "wqT": tile_w(np.ascontiguousarray(wq_c.T)),
            "wkT": tile_w(np.ascontiguousarray(wk_c.T)),
            "wvT": tile_w(np.ascontiguousarray(wv[rows].astype(f32).T)),
            "woT": tile_w(np.ascontiguousarray(wo[rows].astype(f32).T)),
            "cosT": cosT,
            "sinT": sinT,
            "bq": permuted_vec(bqv[rows].astype(f32)),
            "bk": permuted_vec(bkv[rows].astype(f32)),
            "bv": bvv[rows].astype(f16).reshape(1, DC),
            "bo": bov[rows].astype(f32).reshape(HPC, DH),
            "nwq": permuted_vec(nq[rows].astype(f32)),
            "nwk": permuted_vec(nk[rows].astype(f32)),
        })
    return in_maps


def kernel(**inputs):
    global _COMPILED
    if _COMPILED is None:
        _COMPILED = _build()
    nc = _COMPILED
    in_maps = _prep_inputs(**inputs)
    res = run_bass_kernel_spmd(nc, in_maps, core_ids=list(range(N_CORES)))
    out = np.empty((1, S, DIM), dtype=np.float32)
    for core in range(N_CORES):
        out[0, :, core * DC:(core + 1) * DC] = res.results[core]["outT"].T
    return out


# revision 5
# speedup vs baseline: 1.1549x; 1.1549x over previous
"""CPWanSelfAttention on 8 Trainium2 NeuronCores.

Strategy: tensor-parallel over heads (16 heads -> 2 per core), fp16 datapath.
Per core c (heads 2c, 2c+1):
  - qT/kT = wq_c @ hiddenT in transposed-per-head layout [dhead, S], head dims
    host-permuted to [evens..., odds...] so RoPE pair math is half-tile ops.
  - vT natural [S, d] (PV stationary operand).
  - RMS norm: per-core partial sum-of-squares -> AllReduce [1, 2048] per seq
    half -> row-wise sqrt+recip on [1,2048] -> DMA partition-broadcast fp16.
  - Attention split into half-A (key tiles 0-7, available after RoPE of seq
    half 0) and half-B (key tiles 8-15): half-A's ~20us of work bridges the
    second AllReduce's latency so PE never idles on it. Half-A partial PV
    sums spill to SBUF fp16; half-B re-accumulates and finishes.
  - scoresT[k, q] per head with PAIRED key tiles: two 512-row matmuls into
    one [128,1024] PSUM tile, ONE exp per pair (halves ACT instruction
    overhead, the previous ACT bottleneck). Denominator = DVE fp16 adds +
    ones-matmul broadcast; divide at the end.
  - AllGather per-head outputs in fp16 per 512-seq chunk; out projection
    (lag-1) computes the core's 256 output columns; host transposes/concats.
All matmuls fp16 (1 cyc/row, fast weight loads); PSUM accumulates f32.
Measured max |logit| = 7.2 -> exp < 1.4e3 << fp16 max, so no max-subtraction
(faithful to the nki_flash_attention reference, which also omits it).
"""

from contextlib import ExitStack

import numpy as np
import concourse.bass as bass
import concourse.mybir as mybir
import concourse.tile as tile
from concourse import bacc
from concourse.bass_utils import run_bass_kernel_spmd

N_CORES = 8
S = 1992
SP = 2048          # seq padded to multiple of 128 (nki flash attention contract)
DIM = 2048
NHEADS = 16
DH = 128
HPC = NHEADS // N_CORES   # heads per core = 2
DC = DH * HPC             # out dims per core = 256
KT = DIM // 128           # 16 contraction tiles
NCH = SP // 512           # 4 seq chunks of 512
EPS = 1e-6

F32 = mybir.dt.float32
F16 = mybir.dt.float16

_COMPILED = None


def _build(ag_mode='chunk4', repeat=1, stage=4):
    nc = bacc.Bacc("TRN2", target_bir_lowering=False, debug=False,
                   num_devices=N_CORES)

    # ---- DRAM I/O (per-core shards) ----
    hidT = nc.dram_tensor("hidT", [SP // 512, 128, KT, 512], F16, kind="ExternalInput")
    wqT = nc.dram_tensor("wqT", [128, KT, DC], F16, kind="ExternalInput")
    wkT = nc.dram_tensor("wkT", [128, KT, DC], F16, kind="ExternalInput")
    wvT = nc.dram_tensor("wvT", [128, KT, DC], F16, kind="ExternalInput")
    woT = nc.dram_tensor("woT", [128, KT, DC], F16, kind="ExternalInput")
    cosT = nc.dram_tensor("cosT", [DH, SP], F16, kind="ExternalInput")  # [c;c]
    sinT = nc.dram_tensor("sinT", [DH, SP], F16, kind="ExternalInput")  # [-s;s]
    bq = nc.dram_tensor("bq", [HPC, DH], F32, kind="ExternalInput")
    bk = nc.dram_tensor("bk", [HPC, DH], F32, kind="ExternalInput")
    bv = nc.dram_tensor("bv", [1, DC], F16, kind="ExternalInput")
    bo = nc.dram_tensor("bo", [HPC, DH], F32, kind="ExternalInput")
    nwq = nc.dram_tensor("nwq", [HPC, DH], F32, kind="ExternalInput")
    nwk = nc.dram_tensor("nwk", [HPC, DH], F32, kind="ExternalInput")
    outT = nc.dram_tensor("outT", [DC, S], F32, kind="ExternalOutput")

    rg = [list(range(N_CORES))]
    inv_sqrt_dh = 1.0 / float(np.sqrt(DH))

    def emit(tc, top, rep):
        P = lambda nm: f"{nm}_{rep}"
        const = top.enter_context(tc.tile_pool(name=P("const"), bufs=1))
        pv_pool = top.enter_context(tc.tile_pool(name=P("pv_pool"), bufs=1))
        dram = top.enter_context(tc.tile_pool(name=P("dram"), bufs=1, space="DRAM"))

        ones_col = const.tile([128, 1], F16)
        nc.vector.memset(ones_col[:], 1.0)
        ones_sq = const.tile([128, 128], F16)
        nc.vector.memset(ones_sq[:], 1.0)
        ones_row = const.tile([1, 128], F16)
        nc.vector.memset(ones_row[:], 1.0)
        bq_sb = const.tile([128, HPC], F32)
        bk_sb = const.tile([128, HPC], F32)
        bo_sb = const.tile([128, HPC], F32)
        nwq_sb = const.tile([128, HPC], F32)
        nwk_sb = const.tile([128, HPC], F32)
        # DMA issue occupies the issuing engine's queue (only SP/ACT/GpSimd
        # can trigger); spread the preamble so SP isn't a 20us serial
        # bottleneck. GpSimd is idle until the first collective at ~65us.
        nc.gpsimd.dma_start(bq_sb[:], bq[:].rearrange("h p -> p h"))
        nc.gpsimd.dma_start(bk_sb[:], bk[:].rearrange("h p -> p h"))
        bv_sb = const.tile([1, DC], F16)
        eps_sb = const.tile([128, 1], F32)
        nc.vector.memset(eps_sb[:], EPS)
        # RoPE tables + wo loaded up front (consts; overlap phase-1 compute)
        cos_sb = const.tile([DH, SP], F16)
        sin_sb = const.tile([DH, SP], F16)
        nc.scalar.dma_start(cos_sb[:, 0:SP // 2], cosT[:, 0:SP // 2])
        nc.scalar.dma_start(cos_sb[:, SP // 2:SP], cosT[:, SP // 2:SP])
        nc.scalar.dma_start(sin_sb[:, 0:SP // 2], sinT[:, 0:SP // 2])
        nc.scalar.dma_start(sin_sb[:, SP // 2:SP], sinT[:, SP // 2:SP])
        wo_sb = const.tile([128, KT, DC], F16)

        v_sb = pv_pool.tile([128, SP // 128, DC], F16)  # [s%128, s-tile, d]
        late = top.enter_context(tc.tile_pool(name=P("late"), bufs=1))
        qT = [late.tile([128, SP], F16, name=f"qT{h}_{rep}") for h in range(HPC)]
        kTt = [late.tile([128, SP], F16, name=f"kTt{h}_{rep}") for h in range(HPC)]

        # 2 AllReduce halves over seq: half m covers s in [m*1024,(m+1)*1024);
        # within a half: cols [0:1024] = q sumsq, [1024:2048] = k sumsq
        ar_in = [dram.tile([1, SP], F32, name=f"ar_in{m}_{rep}") for m in range(2)]
        ar_out = [dram.tile([1, SP], F32, addr_space="Shared", name=f"ar_out{m}_{rep}")
                  for m in range(2)]
        rbounce = [dram.tile([1, SP], F16, name=f"rb{m}_{rep}") for m in range(2)]

        with ExitStack() as ph123:
            rawp = ph123.enter_context(tc.tile_pool(name=P("rawp"), bufs=1))
            qraw = [rawp.tile([128, SP], F16, name=f"qraw{h}_{rep}") for h in range(HPC)]
            kraw = [rawp.tile([128, SP], F16, name=f"kraw{h}_{rep}") for h in range(HPC)]

            stat = ph123.enter_context(tc.tile_pool(name=P("stat"), bufs=1))
            rstd_bc = [stat.tile([128, SP], F16, name=f"rstdbc{i}_{rep}")
                       for i in range(2)]
            strip = ph123.enter_context(tc.tile_pool(name=P("strip"), bufs=2))
            rwork = ph123.enter_context(tc.tile_pool(name=P("rwork"), bufs=1))

            def rstd_prep(m):
                """rstd = exp(-0.5 * ln(ms + eps)) on a [16,128] reshape of the
                AllReduce row (16 partitions -> cheap DMA), then fp16
                partition-broadcast via a DRAM bounce. Ln and Exp live in the
                same ACT table set as Identity, so the whole kernel needs ONE
                activation-table load - no reload stalls on this chain."""
                srow = strip.tile([16, 128], F32, name="srow")
                nc.sync.dma_start(
                    srow[:], ar_out[m][:].rearrange("o (p c) -> (o p) c", p=16))
                nc.scalar.activation(srow[:], srow[:],
                                     mybir.ActivationFunctionType.Ln,
                                     bias=eps_sb[0:16, :], scale=1.0 / DIM)
                rrow = strip.tile([16, 128], F16, name="rrow")
                nc.scalar.activation(rrow[:], srow[:],
                                     mybir.ActivationFunctionType.Exp,
                                     scale=-0.5)
                nc.sync.dma_start(
                    rbounce[m][:].rearrange("o (p c) -> (o p) c", p=16), rrow[:])
                sj = slice(m * 1024, (m + 1) * 1024)
                for i in (1, 0):                        # k first: rope needs it
                    nc.sync.dma_start(
                        rstd_bc[i][:, sj],
                        rbounce[m][:, i * 1024:(i + 1) * 1024]
                        .partition_broadcast(128))

            def rope_one(raw, dst, rbc, nw, h, m):
                """norm + RoPE for seq half m (1024 wide), all fp16."""
                sj = slice(m * 1024, (m + 1) * 1024)
                xr = rwork.tile([128, 1024], F16, name="xr")
                nc.vector.tensor_mul(xr[:], raw[h][:, sj], rbc[:, sj])
                nc.vector.tensor_scalar_mul(xr[:], xr[:], nw[:, h:h + 1])
                xs = rwork.tile([128, 1024], F16, name="xs")
                nc.vector.tensor_copy(xs[0:64, :], xr[64:128, :])
                nc.vector.tensor_copy(xs[64:128, :], xr[0:64, :])
                nc.vector.tensor_mul(xr[:], xr[:], cos_sb[:, sj])
                nc.vector.tensor_mul(xs[:], xs[:], sin_sb[:, sj])
                nc.vector.tensor_add(dst[h][:, sj], xr[:], xs[:])
                if m == 1:
                    nc.vector.memset(dst[h][:, S:SP], 0.0)

            # ---------- phase 1: QKV + sumsq per 512-chunk, ARs per half ----
            with ExitStack() as ph1:
                wpool = ph1.enter_context(tc.tile_pool(name=P("wpool"), bufs=1))
                hid = ph1.enter_context(tc.tile_pool(name=P("hid"), bufs=3))
                psA = ph1.enter_context(tc.tile_pool(name=P("psA"), bufs=2, space="PSUM"))

                wq_sb = wpool.tile([128, KT, DC], F16)
                wk_sb = wpool.tile([128, KT, DC], F16)
                wv_sb = wpool.tile([128, KT, DC], F16)
                # first-needed operands on otherwise-idle engine queues
                nc.scalar.dma_start(wq_sb[:, 0:KT // 2, :], wqT[:, 0:KT // 2, :])
                nc.scalar.dma_start(wq_sb[:, KT // 2:KT, :], wqT[:, KT // 2:KT, :])
                nc.gpsimd.dma_start(wk_sb[:, 0:KT // 2, :], wkT[:, 0:KT // 2, :])
                nc.gpsimd.dma_start(wk_sb[:, KT // 2:KT, :], wkT[:, KT // 2:KT, :])
                nc.gpsimd.dma_start(wv_sb[:, 0:KT // 2, :], wvT[:, 0:KT // 2, :])
                nc.gpsimd.dma_start(wv_sb[:, KT // 2:KT, :], wvT[:, KT // 2:KT, :])
                nc.gpsimd.dma_start(bo_sb[:], bo[:].rearrange("h p -> p h"))
                nc.gpsimd.dma_start(nwq_sb[:], nwq[:].rearrange("h p -> p h"))
                nc.gpsimd.dma_start(nwk_sb[:], nwk[:].rearrange("h p -> p h"))
                nc.gpsimd.dma_start(bv_sb[:], bv[:])
                nc.gpsimd.dma_start(wo_sb[:, 0:KT // 2, :], woT[:, 0:KT // 2, :])
                nc.gpsimd.dma_start(wo_sb[:, KT // 2:KT, :], woT[:, KT // 2:KT, :])

                def v_block(j, hch):
                    # v: natural layout, hiddenT as stationary; bias K=1 matmul
                    for st in range(4):
                        gst = j * 4 + st
                        pvp = psA.tile([128, DC], F32, name="pvp")
                        for t in range(KT):
                            nc.tensor.matmul(
                                pvp[:], hch[:, t, st * 128:(st + 1) * 128],
                                wv_sb[:, t, :], start=(t == 0), stop=False)
                        nc.tensor.matmul(pvp[:], ones_row[:], bv_sb[:],
                                         start=False, stop=True)
                        if gst == SP // 128 - 1:
                            nc.vector.memset(v_sb[:, gst, :], 0.0)
                            nv = S - (SP // 128 - 1) * 128
                            nc.vector.tensor_copy(v_sb[0:nv, gst, :], pvp[0:nv, :])
                        else:
                            nc.vector.tensor_copy(v_sb[:, gst, :], pvp[:])

                hchs = {}
                for j in range(SP // 512):
                    sj = slice(j * 512, (j + 1) * 512)
                    hch = hid.tile([128, KT, 512], F16, name="hch")
                    hchs[j] = hch
                    nc.sync.dma_start(hch[:, 0:KT // 2, :], hidT[j][:, 0:KT // 2, :])
                    nc.sync.dma_start(hch[:, KT // 2:KT, :], hidT[j][:, KT // 2:KT, :])

                    for (wsb, raw, bias) in ((wq_sb, qraw, bq_sb), (wk_sb, kraw, bk_sb)):
                        for h in range(HPC):
                            pq = psA.tile([128, 512], F32, name="pqk")
                            for t in range(KT):
                                nc.tensor.matmul(
                                    pq[:], wsb[:, t, h * DH:(h + 1) * DH],
                                    hch[:, t, :], start=(t == 0), stop=(t == KT - 1))
                            nc.scalar.activation(
                                raw[h][:, sj], pq[:],
                                mybir.ActivationFunctionType.Identity,
                                bias=bias[:, h:h + 1])

                    # partial sum-of-squares -> AllReduce input strip
                    for idx, raw in ((0, qraw), (1, kraw)):
                        pss = psA.tile([1, 512], F32, name="pss")
                        for h in range(HPC):
                            sq = rwork.tile([128, 512], F16, name="sq")
                            nc.vector.tensor_mul(sq[:], raw[h][:, sj], raw[h][:, sj])
                            nc.tensor.matmul(pss[:], ones_col[:], sq[:],
                                             start=(h == 0), stop=(h == HPC - 1))
                        ssv = rwork.tile([1, 512], F32, name="ssv")
                        nc.vector.tensor_copy(ssv[:], pss[:])
                        m, off = j // 2, (j % 2) * 512
                        nc.sync.dma_start(
                            ar_in[m][:, idx * 1024 + off: idx * 1024 + off + 512],
                            ssv[:])

                    # v for the first two chunks only; chunks 2-3 are deferred
                    # below so their PE work (and half-A's) fills the second
                    # AllReduce's latency window instead of idling on it
                    if j < 2:
                        v_block(j, hch)

                    # issue AllReduce for a finished seq-half; its latency is
                    # bridged by remaining phase-1 chunks (half 0) or by
                    # half-A attention below (half 1)
                    if (j == 1 or j == SP // 512 - 1) and stage >= 2:
                        m = j // 2
                        if ag_mode != 'nocoll':
                            nc.gpsimd.collective_compute(
                                "AllReduce", mybir.AluOpType.add, replica_groups=rg,
                                ins=[ar_in[m][:].opt()], outs=[ar_out[m][:].opt()])
                        else:
                            nc.sync.dma_start(ar_out[m][:], ar_in[m][:])

                    # head-0 rstd+rope for seq half 0 emitted mid-chunk-3 so
                    # the DVE finishes it right as phase-1 PE drains and
                    # half-A attention can start immediately (its deps are
                    # only half-0 tiles; v copies of chunk 3 feed half-B only)
                    if j == SP // 512 - 1 and stage >= 2:
                        rstd_prep(0)
                        rope_one(kraw, kTt, rstd_bc[1], nwk_sb, 0, 0)
                        rope_one(qraw, qT, rstd_bc[0], nwq_sb, 0, 0)

                v_block(2, hchs[2])
                v_block(3, hchs[3])
                if stage >= 2:
                    rope_one(kraw, kTt, rstd_bc[1], nwk_sb, 1, 0)
                    rope_one(qraw, qT, rstd_bc[0], nwq_sb, 1, 0)

            if stage < 3:
                return

            # ------- attention halves + chunked AllGather + projection ------
            with ExitStack() as ph4:
                aw = ph4.enter_context(tc.tile_pool(name=P("aw"), bufs=2))
                oT = [aw.tile([128, SP], F16, name=f"oT{h}_{rep}", bufs=1)
                      for h in range(HPC)]
                accp = ph4.enter_context(tc.tile_pool(name=P("accp"), bufs=1))
                spillp = ph4.enter_context(tc.tile_pool(name=P("spillp"), bufs=1))
                expp = ph4.enter_context(tc.tile_pool(name=P("expp"), bufs=6))
                psC = ph4.enter_context(tc.tile_pool(name=P("psC"), bufs=2, space="PSUM"))
                psD = ph4.enter_context(tc.tile_pool(name=P("psD"), bufs=2, space="PSUM"))

                # per (chunk j, head h) state carried from half-A to half-B
                accs = {}    # fp16 [128, 1024] running exp sums (pair-layout)
                spills = {}  # fp16 [128, 512] half-A partial PV sums

                def att_half(j, h, kp_lo, kp_hi):
                    """Score+exp+PV for key-tile pairs [kp_lo, kp_hi) of
                    chunk j (512 q), head h. Pairs: kp covers kt 2kp, 2kp+1."""
                    sj = slice(j * 512, (j + 1) * 512)
                    first = kp_lo == 0
                    po = psD.tile([128, 512], F32, name="po")
                    if first:
                        acc = accp.tile([128, 1024], F16, name=f"acc{j}{h}_{rep}")
                        accs[(j, h)] = acc
                    else:
                        acc = accs[(j, h)]
                    for kp in range(kp_lo, kp_hi):
                        ps2 = psC.tile([128, 1024], F32, name="ps2")
                        for z in range(2):
                            kt = 2 * kp + z
                            nc.tensor.matmul(
                                ps2[:, z * 512:(z + 1) * 512],
                                kTt[h][:, kt * 128:(kt + 1) * 128],
                                qT[h][:, sj], start=True, stop=True)
                        et = expp.tile([128, 1024], F16, name="et")
                        nc.scalar.activation(et[:], ps2[:],
                                             mybir.ActivationFunctionType.Exp,
                                             scale=inv_sqrt_dh)
                        # running denominator on the otherwise-idle DVE;
                        # emitted before the PV matmuls so the final pair's
                        # add overlaps PE instead of serializing ahead of the
                        # psm matmul at the (j,h) boundary
                        if kp == 0:
                            nc.vector.tensor_copy(acc[:], et[:])
                        else:
                            nc.vector.tensor_add(acc[:], acc[:], et[:])
                        for z in range(2):
                            kt = 2 * kp + z
                            nc.tensor.matmul(
                                po[:], v_sb[:, kt, h * DH:(h + 1) * DH],
                                et[:, z * 512:(z + 1) * 512],
                                start=(kp == kp_lo and z == 0),
                                stop=(kp == kp_hi - 1 and z == 1))
                    return po

                def att_A(j, h):
                    po = att_half(j, h, 0, KT // 4)
                    sp = spillp.tile([128, 512], F16, name=f"sp{j}{h}_{rep}")
                    with nc.allow_low_precision(
                            reason="partial PV sums; fp16 matches datapath"):
                        nc.vector.tensor_copy(sp[:], po[:])
                    spills[(j, h)] = sp

                def att_B(j, h):
                    sj = slice(j * 512, (j + 1) * 512)
                    lo = 0 if (j, h) not in spills else KT // 4
                    po = att_half(j, h, lo, KT // 2)
                    acc = accs[(j, h)]
                    psm = psD.tile([128, 512], F32, name="psm", bufs=1)
                    for z in range(2):
                        nc.tensor.matmul(psm[:], ones_sq[:],
                                         acc[:, z * 512:(z + 1) * 512],
                                         start=(z == 0), stop=(z == 1))
                    rec = aw.tile([128, 512], F32, name="rec")
                    nc.vector.reciprocal(rec[:], psm[:])
                    if (j, h) in spills:
                        pf = aw.tile([128, 512], F32, name="pf")
                        nc.vector.tensor_add(pf[:], po[:], spills[(j, h)][:])
                        nc.vector.tensor_mul(oT[h][:, sj], pf[:], rec[:])
                    else:
                        nc.vector.tensor_mul(oT[h][:, sj], po[:], rec[:])

                agos = []

                def issue_ag(j):
                    sj = slice(j * 512, (j + 1) * 512)
                    agi = dram.tile([DC, 512], F16, name=f"agi{j}_{rep}")
                    ago = dram.tile([DIM, 512], F16, addr_space="Shared",
                                    name=f"ago{j}_{rep}")
                    for h in range(HPC):
                        nc.sync.dma_start(agi[h * DH:(h + 1) * DH, :], oT[h][:, sj])
                    if ag_mode != 'nocoll':
                        nc.gpsimd.collective_compute(
                            "AllGather", mybir.AluOpType.bypass, replica_groups=rg,
                            ins=[agi[:].opt()], outs=[ago[:].opt()])
                    agos.append(ago)

                def project(j):
                    ago = agos[j]
                    och = aw.tile([128, KT, 512], F16, name="och")
                    for q4 in range(4):
                        nc.sync.dma_start(
                            och[:, q4 * 4:(q4 + 1) * 4, :],
                            ago[q4 * 512:(q4 + 1) * 512, :]
                            .rearrange("(t p) s -> p t s", p=128))
                    for h in range(HPC):
                        pout = psD.tile([128, 512], F32, name="pout", bufs=1)
                        for t in range(KT):
                            nc.tensor.matmul(
                                pout[:], wo_sb[:, t, h * DH:(h + 1) * DH],
                                och[:, t, :], start=(t == 0), stop=(t == KT - 1))
                        ot = aw.tile([128, 512], F32, name="ot")
                        nc.scalar.activation(ot[:], pout[:],
                                             mybir.ActivationFunctionType.Identity,
                                             bias=bo_sb[:, h:h + 1])
                        w = min(512, S - j * 512)
                        nc.sync.dma_start(
                            outT[h * DH:(h + 1) * DH, j * 512:j * 512 + w],
                            ot[:, 0:w])

                # half-A: chunks 0,1 (seq-half-0 queries) x key tiles 0-7.
                # Emitted before anything that depends on AllReduce half 1, so
                # every engine stream has AR1-independent work to chew on
                # while AR1 + rstd(1) + rope(m=1) complete.
                for h in range(HPC):
                    for j in (0, 1):
                        att_A(j, h)

                rstd_prep(1)
                for h in range(HPC):
                    rope_one(kraw, kTt, rstd_bc[1], nwk_sb, h, 1)
                for h in range(HPC):
                    rope_one(qraw, qT, rstd_bc[0], nwq_sb, h, 1)

                # half-B j-major so each chunk's AllGather issues as early as
                # possible; projections slot into the ACT-bound attention
                # stream's PE slack (lag-1 wrt AllGather completion)
                do_ag = stage >= 4 or ag_mode == 'nocoll'
                att_B(0, 0)
                att_B(0, 1)
                if do_ag:
                    issue_ag(0)
                att_B(1, 0)
                att_B(1, 1)
                if do_ag:
                    issue_ag(1)
                att_B(2, 0)
                att_B(2, 1)
                if do_ag:
                    issue_ag(2)
                if stage >= 4:
                    project(0)
                att_B(3, 0)
                if stage >= 4:
                    project(1)
                att_B(3, 1)
                if do_ag:
                    issue_ag(3)
                if stage >= 4:
                    project(2)
                    project(3)

    with tile.TileContext(nc) as tc:
        for rep in range(repeat):
            with ExitStack() as top:
                emit(tc, top, rep)

    nc.compile()
    return nc


def _prep_inputs(hidden_states, freqs_cos, freqs_sin, wq, bq, wk, bk, wv, bv,
                 norm_q_w, norm_k_w, wo, bo):
    """Host-side shard + layout prep. Returns in_maps for 8 cores."""
    f32 = np.float32
    f16 = np.float16
    hid = np.ascontiguousarray(np.asarray(hidden_states)[0].T, dtype=f32)
    hidT = np.zeros((DIM, SP), dtype=f32)
    hidT[:, :S] = hid
    # pre-tile to [chunk j, partition p, ktile t, col c]: d = t*128+p, s = j*512+c
    hidT = np.ascontiguousarray(
        hidT.reshape(KT, 128, SP // 512, 512).transpose(2, 1, 0, 3)).astype(f16)

    def tile_w(wT, dt=f16):               # [DIM, DC] -> [128, KT, DC]
        return np.ascontiguousarray(
            wT.reshape(KT, 128, DC).transpose(1, 0, 2)).astype(dt)

    # RoPE tables: c_j[s] = cos[0,s,0,2j], s_j[s] = sin[0,s,0,2j+1]; stack [t;t]
    c = np.asarray(freqs_cos)[0, :, 0, 0::2].astype(f32).T          # [64, S]
    s = np.asarray(freqs_sin)[0, :, 0, 1::2].astype(f32).T          # [64, S]
    cosT = np.zeros((DH, SP), dtype=f32)
    sinT = np.zeros((DH, SP), dtype=f32)
    cosT[0:64, :S] = c
    cosT[64:128, :S] = c
    sinT[0:64, :S] = -s
    sinT[64:128, :S] = s
    cosT = cosT.astype(f16)
    sinT = sinT.astype(f16)

    perm = np.concatenate([np.arange(0, DH, 2), np.arange(1, DH, 2)])
    wq = np.asarray(wq)
    wk = np.asarray(wk)
    wv = np.asarray(wv)
    wo = np.asarray(wo)
    bqv = np.asarray(bq)
    bkv = np.asarray(bk)
    bvv = np.asarray(bv)
    bov = np.asarray(bo)
    nq = np.asarray(norm_q_w)
    nk = np.asarray(norm_k_w)

    in_maps = []
    for core in range(N_CORES):
        rows = slice(core * DC, (core + 1) * DC)

        def permuted(mat_rows):                                     # [DC, DIM]
            blocks = [mat_rows[h * DH:(h + 1) * DH][perm] for h in range(HPC)]
            return np.concatenate(blocks, axis=0)

        def permuted_vec(vec_rows):                                 # [HPC, DH]
            blocks = [vec_rows[h * DH:(h + 1) * DH][perm] for h in range(HPC)]
            return np.stack(blocks, axis=0)

        wq_c = permuted(wq[rows].astype(f32))
        wk_c = permuted(wk[rows].astype(f32))
        in_maps.append({
            "hidT": hidT,
            "wqT": tile_w(np.ascontiguousarray(wq_c.T)),
            "wkT": tile_w(np.ascontiguousarray(wk_c.T)),
            "wvT": tile_w(np.ascontiguousarray(wv[rows].astype(f32).T)),
            "woT": tile_w(np.ascontiguousarray(wo[rows].astype(f32).T)),
            "cosT": cosT,
            "sinT": sinT,
            "bq": permuted_vec(bqv[rows].astype(f32)),
            "bk": permuted_vec(bkv[rows].astype(f32)),
            "bv": bvv[rows].astype(f16).reshape(1, DC),
            "bo": bov[rows].astype(f32).reshape(HPC, DH),
            "nwq": permuted_vec(nq[rows].astype(f32)),
            "nwk": permuted_vec(nk[rows].astype(f32)),
        })
    return in_maps


def kernel(**inputs):
    global _COMPILED
    if _COMPILED is None:
        _COMPILED = _build()
    nc = _COMPILED
    in_maps = _prep_inputs(**inputs)
    res = run_bass_kernel_spmd(nc, in_maps, core_ids=list(range(N_CORES)))
    out = np.empty((1, S, DIM), dtype=np.float32)
    for core in range(N_CORES):
        out[0, :, core * DC:(core + 1) * DC] = res.results[core]["outT"].T
    return out
